# revision 9
# baseline (speedup 1.0000x reference)
"""Trainium2 Bass kernel for nn_Net_89094801588965 (moe_routing).

Data-parallel over batch on 8 NeuronCores. Per-core layout puts features on
SBUF partitions and batch on the free dim, so every layer's output is directly
the next layer's moving operand (no transposes on device).

Math (identical to the reference):
  h  = relu(x @ fc1_w + b) -> relu(@fc2_w+b) -> relu(@fc3_w+b)   [B,256]
  p  = relu(x @ priv_w[task_id] + priv_b[task_id])               [B,256]
  xc = [p, h]                                                    [B,512]
  per-task heads t=0..9: a3[t] = (relu(relu(xc@h1w[t]+b)@h2w[t]+b))@h3w[t]+b
  out[b] = a3[tt[b]][b]

Device-side restructuring:
  - fc1 and the private layer share the input x -> fused into one [784,656]
    matmul (cols 0..255 = private, 256..655 = fc1).
  - head layer 1: all tasks packed as [512, 320] (task t at cols 32t..32t+27,
    zero padded) -> [320, N] activations.
  - head layer 2: block-diagonal [320, 320], 128-aligned diagonal blocks ->
    3 matmuls (tasks 0-3, 4-7, 8-9).
  - head layer 3 + routing: multiply a2 by the per-task one-hot mask (built on
    the host from tt), then contract with the stacked [330, 10] weight whose
    rows 320..329 are h3_b paired with raw one-hot mask rows -- a single
    contraction yields the routed, biased logits.  Masking only at the end is
    exact because relu/bias garbage of non-selected tasks is zeroed there.
  - all big tensors are host-padded to 128-row multiples and loaded with ONE
    dma_start each (descriptor generation is serialized at ~0.6us per DMA).
  - per-chunk head-2/head-3 work is software-pipelined into the next chunk's
    L1/L2/L3 shadow so the PE never waits on ACT/DVE round-trips.
All matmuls run as float32r (full PE rate, ~1e-4 relative error).
"""

import sys

sys.path.insert(0, "/opt/trn_rl_repo")

import numpy as np

import concourse.bass as bass
import concourse.mybir as mybir
import concourse.tile as tile
from concourse import bacc
from concourse.bass_utils import run_bass_kernel_spmd

F32 = mybir.dt.float32
F32R = mybir.dt.float32r
RELU = mybir.ActivationFunctionType.Relu

B = 65536
D = 784
HID = 400
LAT = 256
T = 10
NCLS = 10
NCORES = 8
R = B // NCORES          # rows per core
CH = 512                 # batch columns per chunk
NCH = R // CH            # chunks per core

M1 = LAT + HID           # 656 fused L1 output (private | fc1)
HP = 32                  # per-task padded head width
HT = T * HP              # 320
MSK = HT + T             # 330 masked-contraction rows: [a2-tasks ; one-hot]

_cache = {}


def _ceil_tiles(n):
    full, rem = divmod(n, 128)
    return [128] * full + ([rem] if rem else [])


L1_K = _ceil_tiles(D)            # [128]*6 + [16]
L1_M = _ceil_tiles(M1)           # [128]*5 + [16]
L2_K = _ceil_tiles(HID)          # [128]*3 + [16]
L2_M = _ceil_tiles(HID)
L3_M = _ceil_tiles(LAT)          # [128, 128]
H1_K = _ceil_tiles(2 * LAT)      # [128]*4
H1_M = _ceil_tiles(HT)           # [128, 128, 64]
H3_K = _ceil_tiles(MSK)          # [128, 128, 74]

# bias column layout inside the single [128, 21] bias tensor; the last three
# columns are the per-partition task-index patterns used to build the one-hot
# routing mask on device (value 255 on never-matching rows)
BC_L1, BC_L2, BC_L3, BC_H1, BC_H2, BC_PT = 0, 6, 10, 12, 15, 18
NBC = 21


def _pad128(n):
    return 128 * ((n + 127) // 128)


def _build_program():
    nc = bacc.Bacc("TRN2", target_bir_lowering=False, debug=False,
                   num_devices=NCORES)

    xT_d = nc.dram_tensor("xT", [_pad128(D), R], F32R, kind="ExternalInput")
    tt_d = nc.dram_tensor("ttf", [1, R], F32R, kind="ExternalInput")
    w1_d = nc.dram_tensor("w1", [_pad128(D), M1], F32R, kind="ExternalInput")
    w2_d = nc.dram_tensor("w2", [_pad128(HID), HID], F32R, kind="ExternalInput")
    w3_d = nc.dram_tensor("w3", [_pad128(HID), LAT], F32R, kind="ExternalInput")
    wh1_d = nc.dram_tensor("wh1", [2 * LAT, HT], F32R, kind="ExternalInput")
    wh2_d = nc.dram_tensor("wh2", [3 * 128, 128], F32R, kind="ExternalInput")
    wh3_d = nc.dram_tensor("wh3", [_pad128(MSK), NCLS], F32R,
                           kind="ExternalInput")
    bias_d = nc.dram_tensor("bias", [128, NBC], F32, kind="ExternalInput")
    ones_d = nc.dram_tensor("ones", [1, 128], F32R, kind="ExternalInput")
    out_d = nc.dram_tensor("out", [NCLS, R], F32, kind="ExternalOutput")

    def as3d(dram):
        return dram[:].rearrange("(j p) m -> p j m", p=128)

    with tile.TileContext(nc) as tc:
        with (
            tc.tile_pool(name="wp", bufs=1) as wp,
            tc.tile_pool(name="xp", bufs=2) as xp,
            tc.tile_pool(name="mp", bufs=3) as mpool,
            tc.tile_pool(name="ap", bufs=2) as ap,
            tc.tile_pool(name="op", bufs=2) as op,
            tc.tile_pool(name="ps", bufs=8, space="PSUM") as ps,
        ):
            # ---- resident weights: one DMA per tensor --------------------
            def load_w3d(dram, nk, ncols, tag):
                t = wp.tile([128, nk, ncols], F32R, tag=tag)
                nc.sync.dma_start(t[:], as3d(dram))
                return t

            def load_x_chunk(ci):
                t = xp.tile([128, len(L1_K), CH], F32R, tag="x")
                nc.sync.dma_start(
                    t[:], as3d(xT_d)[:, :, ci * CH:(ci + 1) * CH])
                return t

            # k-tile-granular interleaved load of W1 and chunk-0 x so the
            # first matmul can start after ~0.6 MB instead of ~4 MB
            w1 = wp.tile([128, len(L1_K), M1], F32R, tag="w1")
            x0 = xp.tile([128, len(L1_K), CH], F32R, tag="x")
            w1_3d, x_3d = as3d(w1_d), as3d(xT_d)
            for ki in range(len(L1_K)):
                nc.sync.dma_start(w1[:, ki, :], w1_3d[:, ki, :])
                nc.sync.dma_start(x0[:, ki, :], x_3d[:, ki, 0:CH])
            x1 = load_x_chunk(1)
            w2 = load_w3d(w2_d, len(L2_K), HID, "w2")
            w3 = load_w3d(w3_d, len(L2_K), LAT, "w3")
            wh1 = load_w3d(wh1_d, len(H1_K), HT, "wh1")
            wh2 = load_w3d(wh2_d, 3, 128, "wh2")
            wh3 = load_w3d(wh3_d, len(H3_K), NCLS, "wh3")
            bias = wp.tile([128, NBC], F32, tag="bias")
            nc.sync.dma_start(bias[:], bias_d[:])
            ones = wp.tile([1, 128], F32R, tag="ones")
            nc.sync.dma_start(ones[:], ones_d[:])

            # ---- helpers -------------------------------------------------
            def mm_layer(rhs3, ksizes, w3t, msizes, rhs_list=None):
                """K-accumulated matmuls; rhs3 is a [128, nk, CH] tile or
                rhs_list a list of [kp, CH] tiles."""
                psums = []
                c0 = 0
                for mi, mp_ in enumerate(msizes):
                    pt = ps.tile([mp_, CH], F32, tag="ps")
                    nk = len(ksizes)
                    for ki, kp in enumerate(ksizes):
                        rhs = (rhs3[0:kp, ki, :] if rhs3 is not None
                               else rhs_list[ki][:])
                        nc.tensor.matmul(
                            pt[:], w3t[0:kp, ki, c0:c0 + mp_], rhs,
                            start=(ki == 0), stop=(ki == nk - 1),
                        )
                    psums.append(pt)
                    c0 += mp_
                return psums

            def act_relu(psums, bcol, msizes, tag, eng="act"):
                outs = []
                for mi, mp_ in enumerate(msizes):
                    t = ap.tile([mp_, CH], F32R, tag=f"{tag}{mi}")
                    bap = bias[:mp_, bcol + mi:bcol + mi + 1]
                    if eng == "act":
                        nc.scalar.activation(t[:], psums[mi][:], RELU,
                                             bias=bap, scale=1.0)
                    else:
                        nc.vector.tensor_scalar(
                            t[:], psums[mi][:], bap, 0.0,
                            op0=mybir.AluOpType.add, op1=mybir.AluOpType.max)
                    outs.append(t)
                return outs

            tails_h2 = []
            tails_h3 = []

            for ci in range(NCH):
                cs = ci * CH
                xk = x0 if ci == 0 else (x1 if ci == 1 else load_x_chunk(ci))
                tts = mpool.tile([1, CH], F32R, tag="tt")
                nc.sync.dma_start(tts[:], tt_d[:, cs:cs + CH])

                # L1 fused (private | fc1)
                ps1 = mm_layer(xk, L1_K, w1, L1_M)
                a_l1 = act_relu(ps1, BC_L1, L1_M, "l1o")
                x2 = [a_l1[0], a_l1[1]]
                h1t = [a_l1[2], a_l1[3], a_l1[4], a_l1[5]]

                # previous chunk's head-2 runs inside this chunk's L1 shadow
                while tails_h2:
                    tails_h2.pop(0)()

                ps2 = mm_layer(None, L2_K, w2, L2_M, rhs_list=h1t)
                h2t = act_relu(ps2, BC_L2, L2_M, "l2o", eng="dve")
                ps3 = mm_layer(None, L2_K, w3, L3_M, rhs_list=h2t)
                x2 += act_relu(ps3, BC_L3, L3_M, "l3o", eng="dve")

                # previous chunk's head-3 + store
                while tails_h3:
                    tails_h3.pop(0)()

                ph1 = mm_layer(None, H1_K, wh1, H1_M, rhs_list=x2)
                a1 = act_relu(ph1, BC_H1, H1_M, "a1")

                def tail_h2(a1=a1, tts=tts):
                    # broadcast tt over all 128 partitions with a K=1 matmul
                    bc = ps.tile([128, CH], F32, tag="ps")
                    nc.tensor.matmul(bc[:], ones[:], tts[:],
                                     start=True, stop=True)
                    ph2 = []
                    for i, kp in enumerate(H1_M):
                        pt = ps.tile([kp, CH], F32, tag="ps")
                        nc.tensor.matmul(pt[:], wh2[0:kp, i, 0:kp], a1[i][:],
                                         start=True, stop=True)
                        ph2.append(pt)
                    a2 = act_relu(ph2, BC_H2, H1_M, "a2")
                    # am[i] = (tt == task_of_row) * a2 in one DVE op each;
                    # the 74-row tile also carries the raw one-hot rows that
                    # select h3_b in the head-3 contraction
                    eq, mul = mybir.AluOpType.is_equal, mybir.AluOpType.mult
                    am = []
                    for i in range(2):
                        t = ap.tile([128, CH], F32R, tag=f"am{i}")
                        nc.vector.scalar_tensor_tensor(
                            t[:], bc[:], bias[:, BC_PT + i:BC_PT + i + 1],
                            a2[i][:], op0=eq, op1=mul)
                        am.append(t)
                    t2 = ap.tile([H3_K[2], CH], F32R, tag="am2")
                    nc.vector.scalar_tensor_tensor(
                        t2[0:64, :], bc[0:64, :],
                        bias[0:64, BC_PT + 2:BC_PT + 3],
                        a2[2][:], op0=eq, op1=mul)
                    nc.vector.tensor_scalar(
                        t2[64:, :], bc[64:H3_K[2], :],
                        bias[64:H3_K[2], BC_PT + 2:BC_PT + 3], None, op0=eq)
                    am.append(t2)
                    return am

                def tail_h3(am, cs=cs):
                    po = ps.tile([NCLS, CH], F32, tag="ps")
                    for i, kp in enumerate(H3_K):
                        nc.tensor.matmul(po[:], wh3[0:kp, i, :], am[i][:],
                                         start=(i == 0), stop=(i == 2))
                    ot = op.tile([NCLS, CH], F32, tag="o")
                    nc.scalar.copy(ot[:], po[:])
                    nc.sync.dma_start(out_d[:, cs:cs + CH], ot[:])

                def chain(t2=tail_h2, t3=tail_h3):
                    am = t2()
                    tails_h3.append(lambda: t3(am))

                tails_h2.append(chain)

            while tails_h2:
                tails_h2.pop(0)()
            while tails_h3:
                tails_h3.pop(0)()

    nc.compile()
    return nc


def _prepare_inputs(x_s, tt, task_id,
                    fc1_w, fc1_b, fc2_w, fc2_b, fc3_w, fc3_b,
                    priv_w, priv_b, h1_w, h1_b, h2_w, h2_b, h3_w, h3_b):
    f = np.float32
    task_id = int(task_id)

    x2d = np.asarray(x_s, f).reshape(B, D)
    tt = np.asarray(tt).astype(np.int64).reshape(B)

    w1 = np.zeros((_pad128(D), M1), f)
    w1[:D, :LAT] = np.asarray(priv_w[task_id], f)
    w1[:D, LAT:] = np.asarray(fc1_w, f)
    b1v = np.concatenate([np.asarray(priv_b[task_id], f),
                          np.asarray(fc1_b, f)])
    w2 = np.zeros((_pad128(HID), HID), f)
    w2[:HID] = np.asarray(fc2_w, f)
    w3 = np.zeros((_pad128(HID), LAT), f)
    w3[:HID] = np.asarray(fc3_w, f)
    b2v = np.asarray(fc2_b, f)
    b3v = np.asarray(fc3_b, f)

    wh1 = np.zeros((2 * LAT, HT), f)
    bh1v = np.zeros(HT, f)
    wh2 = np.zeros((3 * 128, 128), f)
    bh2v = np.zeros(HT, f)
    wh3 = np.zeros((_pad128(MSK), NCLS), f)
    for t in range(T):
        c = HP * t
        wh1[:, c:c + 28] = np.asarray(h1_w[t], f)
        bh1v[c:c + 28] = np.asarray(h1_b[t], f)
        blk, off = divmod(c, 128)
        wh2[128 * blk + off:128 * blk + off + 28, off:off + 28] = \
            np.asarray(h2_w[t], f)
        bh2v[c:c + 28] = np.asarray(h2_b[t], f)
        wh3[c:c + 28, :] = np.asarray(h3_w[t], f)
        wh3[HT + t, :] = np.asarray(h3_b[t], f)

    def col_bias(parts):
        out = np.zeros((128, NBC), f)
        col = 0
        for v, msizes in parts:
            r0 = 0
            for mp_ in msizes:
                out[:mp_, col] = v[r0:r0 + mp_]
                r0 += mp_
                col += 1
        return out

    bias = col_bias([(b1v, L1_M), (b2v, L2_M), (b3v, L3_M),
                     (bh1v, H1_M), (bh2v, H1_M)])
    p = np.arange(128)
    bias[:, BC_PT] = p // HP                       # tasks 0..3
    bias[:, BC_PT + 1] = 4 + p // HP               # tasks 4..7
    pt2 = np.full(128, 255.0)
    pt2[:64] = 8 + p[:64] // HP                    # tasks 8..9
    pt2[64:74] = np.arange(T)                      # raw one-hot rows
    bias[:, BC_PT + 2] = pt2

    shared = {"w1": w1, "w2": w2, "w3": w3, "wh1": wh1, "wh2": wh2,
              "wh3": wh3, "bias": bias, "ones": np.ones((1, 128), f)}

    in_maps = []
    for c in range(NCORES):
        sl = slice(c * R, (c + 1) * R)
        xT = np.zeros((_pad128(D), R), f)
        xT[:D] = x2d[sl].T
        m = dict(shared)
        m["xT"] = xT
        m["ttf"] = tt[sl].astype(f).reshape(1, R)
        in_maps.append(m)
    return in_maps


def run(inputs, trace=False, **kw):
    if "nc" not in _cache:
        _cache["nc"] = _build_program()
    nc = _cache["nc"]
    inputs = {k: v for k, v in inputs.items() if k != "x_p"}
    in_maps = _prepare_inputs(**inputs)
    res = run_bass_kernel_spmd(nc, in_maps, list(range(NCORES)),
                               trace=trace, **kw)
    outs = [res.results[c]["out"] for c in range(NCORES)]        # [10, R] each
    full = np.concatenate(outs, axis=1)                          # [10, B]
    return np.ascontiguousarray(full.T), res                     # [B, 10]


def kernel(**inputs):
    out, _ = run(inputs, trace=False)
    return out


# revision 10
# speedup vs baseline: 1.0501x; 1.0501x over previous
"""Trainium2 Bass kernel for nn_Net_89094801588965 (moe_routing).

Data-parallel over batch on 8 NeuronCores. Per-core layout puts features on
SBUF partitions and batch on the free dim, so every layer's output is directly
the next layer's moving operand (no transposes on device).

Math (identical to the reference):
  h  = relu(x @ fc1_w + b) -> relu(@fc2_w+b) -> relu(@fc3_w+b)   [B,256]
  p  = relu(x @ priv_w[task_id] + priv_b[task_id])               [B,256]
  xc = [p, h]                                                    [B,512]
  per-task heads t=0..9: a3[t] = (relu(relu(xc@h1w[t]+b)@h2w[t]+b))@h3w[t]+b
  out[b] = a3[tt[b]][b]

Device-side restructuring:
  - fc1 and the private layer share the input x -> fused into one [784,656]
    matmul (cols 0..255 = private, 256..655 = fc1).
  - head layer 1: all tasks packed as [512, 320] (task t at cols 32t..32t+27,
    zero padded) -> [320, N] activations.
  - head layer 2: block-diagonal [320, 320], 128-aligned diagonal blocks ->
    3 matmuls (tasks 0-3, 4-7, 8-9).
  - head layer 3 + routing: multiply a2 by the per-task one-hot mask (built on
    the host from tt), then contract with the stacked [330, 10] weight whose
    rows 320..329 are h3_b paired with raw one-hot mask rows -- a single
    contraction yields the routed, biased logits.  Masking only at the end is
    exact because relu/bias garbage of non-selected tasks is zeroed there.
  - all big tensors are host-padded to 128-row multiples and loaded with ONE
    dma_start each (descriptor generation is serialized at ~0.6us per DMA).
  - per-chunk head-2/head-3 work is software-pipelined into the next chunk's
    L1/L2/L3 shadow so the PE never waits on ACT/DVE round-trips.
All matmuls run as float32r (full PE rate, ~1e-4 relative error).
"""

import sys

sys.path.insert(0, "/opt/trn_rl_repo")

import numpy as np

import concourse.bass as bass
import concourse.mybir as mybir
import concourse.tile as tile
from concourse import bacc
from concourse.bass_utils import run_bass_kernel_spmd

F32 = mybir.dt.float32
F32R = mybir.dt.float32r
RELU = mybir.ActivationFunctionType.Relu

B = 65536
D = 784
HID = 400
LAT = 256
T = 10
NCLS = 10
NCORES = 8
R = B // NCORES          # rows per core
CH = 512                 # batch columns per chunk
NCH = R // CH            # chunks per core

M1 = LAT + HID           # 656 fused L1 output (private | fc1)
HP = 32                  # per-task padded head width
HT = T * HP              # 320
MSK = HT + T             # 330 masked-contraction rows: [a2-tasks ; one-hot]

_cache = {}


def _ceil_tiles(n):
    full, rem = divmod(n, 128)
    return [128] * full + ([rem] if rem else [])


L1_K = _ceil_tiles(D)            # [128]*6 + [16]
L1_M = _ceil_tiles(M1)           # [128]*5 + [16]
L2_K = _ceil_tiles(HID)          # [128]*3 + [16]
L2_M = _ceil_tiles(HID)
L3_M = _ceil_tiles(LAT)          # [128, 128]
H1_K = _ceil_tiles(2 * LAT)      # [128]*4
H1_M = _ceil_tiles(HT)           # [128, 128, 64]
H3_K = _ceil_tiles(MSK)          # [128, 128, 74]

# bias column layout inside the single [128, 21] bias tensor; the last three
# columns are the per-partition task-index patterns used to build the one-hot
# routing mask on device (value 255 on never-matching rows)
BC_L1, BC_L2, BC_L3, BC_H1, BC_H2, BC_PT = 0, 6, 10, 12, 15, 18
NBC = 21


def _pad128(n):
    return 128 * ((n + 127) // 128)


def _build_program():
    nc = bacc.Bacc("TRN2", target_bir_lowering=False, debug=False,
                   num_devices=NCORES)

    xT_d = nc.dram_tensor("xT", [D, R], F32R, kind="ExternalInput")
    tt_d = nc.dram_tensor("ttf", [1, R], F32R, kind="ExternalInput")
    w1_d = nc.dram_tensor("w1", [D, M1], F32R, kind="ExternalInput")
    w2_d = nc.dram_tensor("w2", [HID, HID], F32R, kind="ExternalInput")
    w3_d = nc.dram_tensor("w3", [HID, LAT], F32R, kind="ExternalInput")
    wh1_d = nc.dram_tensor("wh1", [2 * LAT, HT], F32R, kind="ExternalInput")
    wh2_d = nc.dram_tensor("wh2", [3 * 128, 128], F32R, kind="ExternalInput")
    wh3_d = nc.dram_tensor("wh3", [MSK, NCLS], F32R, kind="ExternalInput")
    bias_d = nc.dram_tensor("bias", [128, NBC], F32, kind="ExternalInput")
    ones_d = nc.dram_tensor("ones", [1, 128], F32R, kind="ExternalInput")
    out_d = nc.dram_tensor("out", [NCLS, R], F32, kind="ExternalOutput")

    def as3d(dram):
        return dram[:].rearrange("(j p) m -> p j m", p=128)

    with tile.TileContext(nc) as tc:
        with (
            tc.tile_pool(name="wp", bufs=1) as wp,
            tc.tile_pool(name="xp", bufs=2) as xp,
            tc.tile_pool(name="mp", bufs=3) as mpool,
            tc.tile_pool(name="ap", bufs=2) as ap,
            tc.tile_pool(name="op", bufs=2) as op,
            tc.tile_pool(name="ps", bufs=8, space="PSUM") as ps,
        ):
            # ---- resident weights: minimal DMA count, exact byte counts;
            # padded tail rows are never transferred (tiles are zero-filled
            # only where a matmul would read them, which is nowhere: k-tails
            # use [0:kp] slices)
            def load_w3d(dram, ksizes, ncols, tag, pool=wp, col0=0):
                nk, kt = len(ksizes), ksizes[-1]
                t = pool.tile([128, nk, ncols], F32R, tag=tag)
                nfull = nk - (1 if kt < 128 else 0)
                src_ = dram[0:128 * nfull, col0:col0 + ncols].rearrange(
                    "(j p) m -> p j m", p=128)
                nc.sync.dma_start(t[:, 0:nfull, :], src_)
                if kt < 128:
                    nc.sync.dma_start(
                        t[0:kt, nk - 1, :],
                        dram[128 * nfull:128 * nfull + kt,
                             col0:col0 + ncols])
                return t

            def load_x_chunk(ci):
                return load_w3d(xT_d, L1_K, CH, "x", pool=xp, col0=ci * CH)

            # chunk-0 x + W1 interleaved at k-tile granularity so the first
            # matmuls start after ~0.6 MB; bias/ones right behind them so
            # chunk-0 activations are never blocked
            w1 = wp.tile([128, len(L1_K), M1], F32R, tag="w1")
            x0 = xp.tile([128, len(L1_K), CH], F32R, tag="x")
            for ki, kp in enumerate(L1_K):
                nc.sync.dma_start(w1[0:kp, ki, :],
                                  w1_d[128 * ki:128 * ki + kp, :])
                nc.sync.dma_start(x0[0:kp, ki, :],
                                  xT_d[128 * ki:128 * ki + kp, 0:CH])
                if ki == 0:
                    bias = wp.tile([128, NBC], F32, tag="bias")
                    nc.sync.dma_start(bias[:], bias_d[:])
                    ones = wp.tile([1, 128], F32R, tag="ones")
                    nc.sync.dma_start(ones[:], ones_d[:])
            w2 = load_w3d(w2_d, L2_K, HID, "w2")
            x1 = load_x_chunk(1)
            w3 = load_w3d(w3_d, L2_K, LAT, "w3")
            wh1 = load_w3d(wh1_d, H1_K, HT, "wh1")
            wh2 = load_w3d(wh2_d, [128] * 3, 128, "wh2")
            wh3 = load_w3d(wh3_d, H3_K, NCLS, "wh3")

            # ---- helpers -------------------------------------------------
            def mm_layer(rhs3, ksizes, w3t, msizes, rhs_list=None):
                """K-accumulated matmuls; rhs3 is a [128, nk, CH] tile or
                rhs_list a list of [kp, CH] tiles."""
                psums = []
                c0 = 0
                for mi, mp_ in enumerate(msizes):
                    pt = ps.tile([mp_, CH], F32, tag="ps")
                    nk = len(ksizes)
                    for ki, kp in enumerate(ksizes):
                        rhs = (rhs3[0:kp, ki, :] if rhs3 is not None
                               else rhs_list[ki][:])
                        nc.tensor.matmul(
                            pt[:], w3t[0:kp, ki, c0:c0 + mp_], rhs,
                            start=(ki == 0), stop=(ki == nk - 1),
                        )
                    psums.append(pt)
                    c0 += mp_
                return psums

            def act_relu(psums, bcol, msizes, tag, eng="act"):
                outs = []
                for mi, mp_ in enumerate(msizes):
                    t = ap.tile([mp_, CH], F32R, tag=f"{tag}{mi}")
                    bap = bias[:mp_, bcol + mi:bcol + mi + 1]
                    if eng == "act":
                        nc.scalar.activation(t[:], psums[mi][:], RELU,
                                             bias=bap, scale=1.0)
                    else:
                        nc.vector.tensor_scalar(
                            t[:], psums[mi][:], bap, 0.0,
                            op0=mybir.AluOpType.add, op1=mybir.AluOpType.max)
                    outs.append(t)
                return outs

            tails_h2 = []
            tails_h3 = []

            for ci in range(NCH):
                cs = ci * CH
                xk = x0 if ci == 0 else (x1 if ci == 1 else load_x_chunk(ci))
                tts = mpool.tile([1, CH], F32R, tag="tt")
                nc.sync.dma_start(tts[:], tt_d[:, cs:cs + CH])

                # L1 fused (private | fc1)
                ps1 = mm_layer(xk, L1_K, w1, L1_M)
                a_l1 = act_relu(ps1, BC_L1, L1_M, "l1o")
                x2 = [a_l1[0], a_l1[1]]
                h1t = [a_l1[2], a_l1[3], a_l1[4], a_l1[5]]

                # previous chunk's head-2 runs inside this chunk's L1 shadow
                while tails_h2:
                    tails_h2.pop(0)()

                ps2 = mm_layer(None, L2_K, w2, L2_M, rhs_list=h1t)
                h2t = act_relu(ps2, BC_L2, L2_M, "l2o", eng="dve")
                ps3 = mm_layer(None, L2_K, w3, L3_M, rhs_list=h2t)
                x2 += act_relu(ps3, BC_L3, L3_M, "l3o", eng="dve")

                # previous chunk's head-3 + store
                while tails_h3:
                    tails_h3.pop(0)()

                ph1 = mm_layer(None, H1_K, wh1, H1_M, rhs_list=x2)
                a1 = act_relu(ph1, BC_H1, H1_M, "a1")

                def tail_h2(a1=a1, tts=tts):
                    # broadcast tt over all 128 partitions with a K=1 matmul
                    bc = ps.tile([128, CH], F32, tag="ps")
                    nc.tensor.matmul(bc[:], ones[:], tts[:],
                                     start=True, stop=True)
                    ph2 = []
                    for i, kp in enumerate(H1_M):
                        pt = ps.tile([kp, CH], F32, tag="ps")
                        nc.tensor.matmul(pt[:], wh2[0:kp, i, 0:kp], a1[i][:],
                                         start=True, stop=True)
                        ph2.append(pt)
                    a2 = act_relu(ph2, BC_H2, H1_M, "a2")
                    # am[i] = (tt == task_of_row) * a2 in one DVE op each;
                    # the 74-row tile also carries the raw one-hot rows that
                    # select h3_b in the head-3 contraction
                    eq, mul = mybir.AluOpType.is_equal, mybir.AluOpType.mult
                    am = []
                    for i in range(2):
                        t = ap.tile([128, CH], F32R, tag=f"am{i}")
                        nc.vector.scalar_tensor_tensor(
                            t[:], bc[:], bias[:, BC_PT + i:BC_PT + i + 1],
                            a2[i][:], op0=eq, op1=mul)
                        am.append(t)
                    t2 = ap.tile([H3_K[2], CH], F32R, tag="am2")
                    nc.vector.scalar_tensor_tensor(
                        t2[0:64, :], bc[0:64, :],
                        bias[0:64, BC_PT + 2:BC_PT + 3],
                        a2[2][:], op0=eq, op1=mul)
                    nc.vector.tensor_scalar(
                        t2[64:, :], bc[64:H3_K[2], :],
                        bias[64:H3_K[2], BC_PT + 2:BC_PT + 3], None, op0=eq)
                    am.append(t2)
                    return am

                def tail_h3(am, cs=cs):
                    po = ps.tile([NCLS, CH], F32, tag="ps")
                    for i, kp in enumerate(H3_K):
                        nc.tensor.matmul(po[:], wh3[0:kp, i, :], am[i][:],
                                         start=(i == 0), stop=(i == 2))
                    ot = op.tile([NCLS, CH], F32, tag="o")
                    nc.scalar.copy(ot[:], po[:])
                    nc.sync.dma_start(out_d[:, cs:cs + CH], ot[:])

                def chain(t2=tail_h2, t3=tail_h3):
                    am = t2()
                    tails_h3.append(lambda: t3(am))

                tails_h2.append(chain)

            while tails_h2:
                tails_h2.pop(0)()
            while tails_h3:
                tails_h3.pop(0)()

    nc.compile()
    return nc


def _prepare_inputs(x_s, tt, task_id,
                    fc1_w, fc1_b, fc2_w, fc2_b, fc3_w, fc3_b,
                    priv_w, priv_b, h1_w, h1_b, h2_w, h2_b, h3_w, h3_b):
    f = np.float32
    task_id = int(task_id)

    x2d = np.asarray(x_s, f).reshape(B, D)
    tt = np.asarray(tt).astype(np.int64).reshape(B)

    w1 = np.concatenate([np.asarray(priv_w[task_id], f),
                         np.asarray(fc1_w, f)], axis=1)
    b1v = np.concatenate([np.asarray(priv_b[task_id], f),
                          np.asarray(fc1_b, f)])
    w2 = np.ascontiguousarray(np.asarray(fc2_w, f))
    w3 = np.ascontiguousarray(np.asarray(fc3_w, f))
    b2v = np.asarray(fc2_b, f)
    b3v = np.asarray(fc3_b, f)

    wh1 = np.zeros((2 * LAT, HT), f)
    bh1v = np.zeros(HT, f)
    wh2 = np.zeros((3 * 128, 128), f)
    bh2v = np.zeros(HT, f)
    wh3 = np.zeros((MSK, NCLS), f)
    for t in range(T):
        c = HP * t
        wh1[:, c:c + 28] = np.asarray(h1_w[t], f)
        bh1v[c:c + 28] = np.asarray(h1_b[t], f)
        blk, off = divmod(c, 128)
        wh2[128 * blk + off:128 * blk + off + 28, off:off + 28] = \
            np.asarray(h2_w[t], f)
        bh2v[c:c + 28] = np.asarray(h2_b[t], f)
        wh3[c:c + 28, :] = np.asarray(h3_w[t], f)
        wh3[HT + t, :] = np.asarray(h3_b[t], f)

    def col_bias(parts):
        out = np.zeros((128, NBC), f)
        col = 0
        for v, msizes in parts:
            r0 = 0
            for mp_ in msizes:
                out[:mp_, col] = v[r0:r0 + mp_]
                r0 += mp_
                col += 1
        return out

    bias = col_bias([(b1v, L1_M), (b2v, L2_M), (b3v, L3_M),
                     (bh1v, H1_M), (bh2v, H1_M)])
    p = np.arange(128)
    bias[:, BC_PT] = p // HP                       # tasks 0..3
    bias[:, BC_PT + 1] = 4 + p // HP               # tasks 4..7
    pt2 = np.full(128, 255.0)
    pt2[:64] = 8 + p[:64] // HP                    # tasks 8..9
    pt2[64:74] = np.arange(T)                      # raw one-hot rows
    bias[:, BC_PT + 2] = pt2

    shared = {"w1": w1, "w2": w2, "w3": w3, "wh1": wh1, "wh2": wh2,
              "wh3": wh3, "bias": bias, "ones": np.ones((1, 128), f)}

    in_maps = []
    for c in range(NCORES):
        sl = slice(c * R, (c + 1) * R)
        xT = np.ascontiguousarray(x2d[sl].T)
        m = dict(shared)
        m["xT"] = xT
        m["ttf"] = tt[sl].astype(f).reshape(1, R)
        in_maps.append(m)
    return in_maps


def run(inputs, trace=False, **kw):
    if "nc" not in _cache:
        _cache["nc"] = _build_program()
    nc = _cache["nc"]
    inputs = {k: v for k, v in inputs.items() if k != "x_p"}
    in_maps = _prepare_inputs(**inputs)
    res = run_bass_kernel_spmd(nc, in_maps, list(range(NCORES)),
                               trace=trace, **kw)
    outs = [res.results[c]["out"] for c in range(NCORES)]        # [10, R] each
    full = np.concatenate(outs, axis=1)                          # [10, B]
    return np.ascontiguousarray(full.T), res                     # [B, 10]


def kernel(**inputs):
    out, _ = run(inputs, trace=False)
    return out
